# revision 3
# baseline (speedup 1.0000x reference)
"""BottomPool (cummax along H) for (16, 256, 128, 128) f32 on 8 TRN2 NeuronCores.

Sharding: data-parallel over batch — 2 batches per core. Each core's shard
is viewed as [512 slabs, H*W] where a slab is one (b, c) image of
H=128 x W=128 f32 stored h-major. Partition dim = slab; the cummax along h
runs on the free axis as a two-level blocked scan:
  view free as (B blocks, S h-slices, W), B*S = H
  step1: in-block sequential scan over S   (S-1 vector ops, all blocks at once)
  step2: scan of block-last slices across B (B-1 small ops)
  step3: distribute prev-block prefix into slices 0..S-2 (S-1 ops)
~1.8 vector passes over the data, overlapped with >=1MiB DMAs.

walrus codegen accepts only a small number of sync waits per instruction
(one for DMA pseudo-instructions), but Tile's sem assigner is not
transitively minimal and can attach more. strip_implied_waits() removes
waits that are provably implied: a wait is redundant when the kept waits'
completion closure (instructions that must have completed, including
same-HWDGE-ring FIFO predecessors of completed DMAs) already forces the
waited semaphore to the required value.
"""

import numpy as np

from concourse import bass, mybir, tile
from concourse.bass_utils import run_bass_kernel_spmd

N_CORES = 8
BATCH, CH, H, W = 16, 256, 128, 128
FREE = H * W
P = 128                      # slabs per tile = SBUF partitions
SLABS = (BATCH // N_CORES) * CH  # 512 slabs per core

_NC_CACHE = {}


def _strip_instruction_waits(nc, max_waits={"InstDMACopy": 1}):
    insts = []
    for f in nc.m.functions:
        for b in f.blocks:
            insts.extend(b.instructions)

    # Monotone-sem updater table: sem_id -> [(cum_value_after, inst_idx)].
    # Sems touched by non-monotone updates are excluded entirely.
    poisoned = set()
    cum = {}
    updaters = {}
    inst_updates = [[] for _ in insts]  # idx -> [(sem_id, cum_after)]
    for idx, ins in enumerate(insts):
        si = ins.sync_info
        if si is None:
            continue
        for u in si.on_update:
            if u.update_mode == "sem-add-imm" and u.update_reg is None:
                val = u.update_value
            elif u.update_mode == "sem-inc":
                val = 1
            else:
                poisoned.add(u.id)
                continue
            cum[u.id] = cum.get(u.id, 0) + val
            updaters.setdefault(u.id, []).append((cum[u.id], idx))
            inst_updates[idx].append((u.id, cum[u.id]))

    # Same-HWDGE-ring FIFO order: DMAs issued on one ring complete in
    # program order, so a later DMA's completion implies all earlier ones.
    ring_pos = {}   # inst_idx -> (queue, position)
    ring_members = {}  # queue -> [inst_idx in order]
    for idx, ins in enumerate(insts):
        if isinstance(ins, mybir.InstDMACopy):
            q = ins.queue
            ring_members.setdefault(q, []).append(idx)
            ring_pos[idx] = (q, len(ring_members[q]) - 1)

    inst_waits = []
    for ins in insts:
        si = ins.sync_info
        ws = []
        if si is not None:
            for w in si.on_wait:
                if w.wait_mode == "sem-ge-imm" and w.wait_reg is None:
                    ws.append((w.id, w.wait_value, True))
                else:
                    ws.append((w.id, w.wait_value, False))
        inst_waits.append(ws)

    def facts_from(seed_waits):
        """Fixpoint: semaphore lower bounds guaranteed once seed_waits hold."""
        facts = {}
        for sid, v, clean in seed_waits:
            if clean and sid not in poisoned:
                facts[sid] = max(facts.get(sid, 0), v)
        completed = set()
        changed = True
        while changed:
            changed = False
            for sid, v in list(facts.items()):
                for cval, idx in updaters.get(sid, []):
                    if cval > v:
                        break
                    if idx not in completed:
                        completed.add(idx)
                        changed = True
            for idx in list(completed):
                rp = ring_pos.get(idx)
                if rp is not None:
                    q, pos = rp
                    for pidx in ring_members[q][:pos]:
                        if pidx not in completed:
                            completed.add(pidx)
                            changed = True
            for idx in list(completed):
                for sid, v, clean in inst_waits[idx]:
                    if clean and sid not in poisoned and facts.get(sid, 0) < v:
                        facts[sid] = v
                        changed = True
                for sid, cval in inst_updates[idx]:
                    if sid not in poisoned and facts.get(sid, 0) < cval:
                        facts[sid] = cval
                        changed = True
        return facts

    n_stripped = 0
    for idx, ins in enumerate(insts):
        si = ins.sync_info
        if si is None or len(si.on_wait) <= 1:
            continue
        kept = list(si.on_wait)

        def key(w):
            return (w.id, w.wait_value, w.wait_mode == "sem-ge-imm" and w.wait_reg is None)

        progress = True
        while len(kept) > 1 and progress:
            progress = False
            for w in list(kept):
                sid, v, clean = key(w)
                if not clean or sid in poisoned:
                    continue
                others = [key(k) for k in kept if k is not w]
                if facts_from(others).get(sid, 0) >= v:
                    kept.remove(w)
                    n_stripped += 1
                    progress = True
                    break
        limit = max_waits.get(type(ins).__name__)
        if limit is not None and len(kept) > limit:
            raise RuntimeError(
                f"{type(ins).__name__} {ins.name} still has {len(kept)} waits: "
                f"{[(w.ant_name, w.wait_value) for w in kept]}"
            )
        if len(kept) != len(si.on_wait):
            ins.sync_info = mybir.SyncInfo(on_wait=kept, on_update=list(si.on_update))
    return n_stripped


def build_nc(n_slabs: int = SLABS, bufs: int = 3, blocks: int = 16):
    B = blocks
    S = H // B
    assert n_slabs % P == 0
    n_tiles = n_slabs // P

    nc = bass.Bass("TRN2", target_bir_lowering=False, debug=False)
    x = nc.dram_tensor("x", [n_slabs, FREE], mybir.dt.float32, kind="ExternalInput").ap()
    out = nc.dram_tensor("out", [n_slabs, FREE], mybir.dt.float32, kind="ExternalOutput").ap()

    with tile.TileContext(nc) as tc:
        with tc.tile_pool(name="work", bufs=bufs) as pool:
            for t in range(n_tiles):
                tl = pool.tile([P, FREE], mybir.dt.float32)
                # loads on the SP HWDGE ring, stores on the ACT ring: each
                # ring is FIFO, letting the wait-stripper collapse the
                # kernel-tail drain to a single wait on the last store.
                nc.sync.dma_start(tl[:], x[t * P:(t + 1) * P, :])
                v = tl[:].rearrange("p (b s w) -> p b s w", b=B, s=S, w=W)
                for j in range(1, S):
                    nc.vector.tensor_max(v[:, :, j, :], v[:, :, j, :], v[:, :, j - 1, :])
                for b in range(1, B):
                    nc.vector.tensor_max(v[:, b, S - 1, :], v[:, b, S - 1, :], v[:, b - 1, S - 1, :])
                for j in range(S - 1):
                    nc.vector.tensor_max(v[:, 1:, j, :], v[:, 1:, j, :], v[:, 0:B - 1, S - 1, :])
                nc.scalar.dma_start(out[t * P:(t + 1) * P, :], tl[:])

    _strip_instruction_waits(nc)
    return nc


def _get_nc():
    key = "default"
    if key not in _NC_CACHE:
        _NC_CACHE[key] = build_nc()
    return _NC_CACHE[key]


def _shard(x: np.ndarray):
    per = BATCH // N_CORES
    return [
        np.ascontiguousarray(x[i * per:(i + 1) * per]).reshape(SLABS, FREE)
        for i in range(N_CORES)
    ]


def _unshard(outs):
    per = BATCH // N_CORES
    return np.concatenate(
        [o.reshape(per, CH, H, W) for o in outs], axis=0
    )


def run(x: np.ndarray, trace: bool = False, **kwargs):
    """Run on hardware; returns (full_output, BassKernelResults)."""
    x = np.asarray(x, dtype=np.float32)
    assert x.shape == (BATCH, CH, H, W), x.shape
    in_maps = [{"x": s} for s in _shard(x)]
    nc = _get_nc()
    res = run_bass_kernel_spmd(
        nc, in_maps, core_ids=list(range(N_CORES)), trace=trace, **kwargs
    )
    out = _unshard([res.results[i]["out"] for i in range(N_CORES)])
    return out, res


def kernel(x) -> np.ndarray:
    out, _ = run(np.asarray(x), trace=False)
    return out


# revision 4
# speedup vs baseline: 1.0519x; 1.0519x over previous
"""BottomPool (cummax along H) for (16, 256, 128, 128) f32 on 8 TRN2 NeuronCores.

Sharding: data-parallel over batch — 2 batches per core. Each core's shard
is viewed as [512 slabs, H*W] where a slab is one (b, c) image of
H=128 x W=128 f32 stored h-major. Partition dim = slab; the cummax along h
runs on the free axis as a two-level blocked scan:
  view free as (B blocks, S h-slices, W), B*S = H
  step1: in-block sequential scan over S   (S-1 vector ops, all blocks at once)
  step2: scan of block-last slices across B (B-1 small ops)
  step3: distribute prev-block prefix into slices 0..S-2 (S-1 ops)
~1.8 vector passes over the data, overlapped with >=1MiB DMAs.

walrus codegen accepts only a small number of sync waits per instruction
(one for DMA pseudo-instructions), but Tile's sem assigner is not
transitively minimal and can attach more. strip_implied_waits() removes
waits that are provably implied: a wait is redundant when the kept waits'
completion closure (instructions that must have completed, including
same-HWDGE-ring FIFO predecessors of completed DMAs) already forces the
waited semaphore to the required value.
"""

import numpy as np

from concourse import bass, mybir, tile
from concourse.bass_utils import run_bass_kernel_spmd

N_CORES = 8
BATCH, CH, H, W = 16, 256, 128, 128
FREE = H * W
P = 128                      # slabs per tile = SBUF partitions
SLABS = (BATCH // N_CORES) * CH  # 512 slabs per core

_NC_CACHE = {}


def _strip_instruction_waits(nc, max_waits={"InstDMACopy": 1}):
    insts = []
    for f in nc.m.functions:
        for b in f.blocks:
            insts.extend(b.instructions)

    # Monotone-sem updater table: sem_id -> [(cum_value_after, inst_idx)].
    # Sems touched by non-monotone updates are excluded entirely.
    poisoned = set()
    cum = {}
    updaters = {}
    inst_updates = [[] for _ in insts]  # idx -> [(sem_id, cum_after)]
    for idx, ins in enumerate(insts):
        si = ins.sync_info
        if si is None:
            continue
        for u in si.on_update:
            if u.update_mode == "sem-add-imm" and u.update_reg is None:
                val = u.update_value
            elif u.update_mode == "sem-inc":
                val = 1
            else:
                poisoned.add(u.id)
                continue
            cum[u.id] = cum.get(u.id, 0) + val
            updaters.setdefault(u.id, []).append((cum[u.id], idx))
            inst_updates[idx].append((u.id, cum[u.id]))

    # Same-HWDGE-ring FIFO order: DMAs issued on one ring complete in
    # program order, so a later DMA's completion implies all earlier ones.
    ring_pos = {}   # inst_idx -> (queue, position)
    ring_members = {}  # queue -> [inst_idx in order]
    for idx, ins in enumerate(insts):
        if isinstance(ins, mybir.InstDMACopy):
            q = ins.queue
            ring_members.setdefault(q, []).append(idx)
            ring_pos[idx] = (q, len(ring_members[q]) - 1)

    inst_waits = []
    for ins in insts:
        si = ins.sync_info
        ws = []
        if si is not None:
            for w in si.on_wait:
                if w.wait_mode == "sem-ge-imm" and w.wait_reg is None:
                    ws.append((w.id, w.wait_value, True))
                else:
                    ws.append((w.id, w.wait_value, False))
        inst_waits.append(ws)

    def facts_from(seed_waits):
        """Fixpoint: semaphore lower bounds guaranteed once seed_waits hold."""
        facts = {}
        for sid, v, clean in seed_waits:
            if clean and sid not in poisoned:
                facts[sid] = max(facts.get(sid, 0), v)
        completed = set()
        changed = True
        while changed:
            changed = False
            for sid, v in list(facts.items()):
                for cval, idx in updaters.get(sid, []):
                    if cval > v:
                        break
                    if idx not in completed:
                        completed.add(idx)
                        changed = True
            for idx in list(completed):
                rp = ring_pos.get(idx)
                if rp is not None:
                    q, pos = rp
                    for pidx in ring_members[q][:pos]:
                        if pidx not in completed:
                            completed.add(pidx)
                            changed = True
            for idx in list(completed):
                for sid, v, clean in inst_waits[idx]:
                    if clean and sid not in poisoned and facts.get(sid, 0) < v:
                        facts[sid] = v
                        changed = True
                for sid, cval in inst_updates[idx]:
                    if sid not in poisoned and facts.get(sid, 0) < cval:
                        facts[sid] = cval
                        changed = True
        return facts

    n_stripped = 0
    for idx, ins in enumerate(insts):
        si = ins.sync_info
        if si is None or len(si.on_wait) <= 1:
            continue
        kept = list(si.on_wait)

        def key(w):
            return (w.id, w.wait_value, w.wait_mode == "sem-ge-imm" and w.wait_reg is None)

        progress = True
        while len(kept) > 1 and progress:
            progress = False
            for w in list(kept):
                sid, v, clean = key(w)
                if not clean or sid in poisoned:
                    continue
                others = [key(k) for k in kept if k is not w]
                if facts_from(others).get(sid, 0) >= v:
                    kept.remove(w)
                    n_stripped += 1
                    progress = True
                    break
        limit = max_waits.get(type(ins).__name__)
        if limit is not None and len(kept) > limit:
            raise RuntimeError(
                f"{type(ins).__name__} {ins.name} still has {len(kept)} waits: "
                f"{[(w.ant_name, w.wait_value) for w in kept]}"
            )
        if len(kept) != len(si.on_wait):
            ins.sync_info = mybir.SyncInfo(on_wait=kept, on_update=list(si.on_update))
    return n_stripped


def build_nc(n_slabs: int = SLABS, bufs: int = 3, blocks: int = 16, halves: int = 2,
             alt_load_rings: bool = True):
    """halves: split each tile's load/compute/store into this many h-chunks
    (pipelines at finer granularity; the cross-chunk carry stays inside the
    tile with regular strided APs). Loads alternate the SP/Pool DMA rings,
    stores all ride the ACT ring so its FIFO collapses the tail drain to
    one wait."""
    B = blocks
    S = H // B
    assert n_slabs % P == 0
    assert B % halves == 0
    n_tiles = n_slabs // P
    BH = B // halves           # blocks per half-chunk
    CH_FREE = FREE // halves   # free elems per chunk

    nc = bass.Bass("TRN2", target_bir_lowering=False, debug=False)
    x = nc.dram_tensor("x", [n_slabs, FREE], mybir.dt.float32, kind="ExternalInput").ap()
    out = nc.dram_tensor("out", [n_slabs, FREE], mybir.dt.float32, kind="ExternalOutput").ap()

    load_engines = [nc.sync, nc.gpsimd] if alt_load_rings else [nc.sync]
    n_load = 0

    with tile.TileContext(nc) as tc:
        with tc.tile_pool(name="work", bufs=bufs) as pool:
            for t in range(n_tiles):
                tl = pool.tile([P, FREE], mybir.dt.float32)
                v = tl[:].rearrange("p (b s w) -> p b s w", b=B, s=S, w=W)
                xrow = x[t * P:(t + 1) * P, :]
                orow = out[t * P:(t + 1) * P, :]
                for h in range(halves):
                    b0, b1 = h * BH, (h + 1) * BH
                    c0, c1 = h * CH_FREE, (h + 1) * CH_FREE
                    eng = load_engines[n_load % len(load_engines)]
                    n_load += 1
                    eng.dma_start(tl[:, c0:c1], xrow[:, c0:c1])
                    # step1: in-block scan, all blocks of this chunk at once
                    for j in range(1, S):
                        nc.vector.tensor_max(
                            v[:, b0:b1, j, :], v[:, b0:b1, j, :], v[:, b0:b1, j - 1, :])
                    # step2: chain block-last slices (carry crosses chunk
                    # boundary through block b0-1 inside the same tile)
                    for b in range(b0 if h else 1, b1):
                        nc.vector.tensor_max(
                            v[:, b, S - 1, :], v[:, b, S - 1, :], v[:, b - 1, S - 1, :])
                    # step3: distribute prev-block prefix into slices 0..S-2
                    p0 = b0 if h else 1
                    for j in range(S - 1):
                        nc.vector.tensor_max(
                            v[:, p0:b1, j, :], v[:, p0:b1, j, :],
                            v[:, p0 - 1:b1 - 1, S - 1, :])
                    nc.scalar.dma_start(orow[:, c0:c1], tl[:, c0:c1])

    _strip_instruction_waits(nc)
    return nc


def _get_nc():
    key = "default"
    if key not in _NC_CACHE:
        _NC_CACHE[key] = build_nc()
    return _NC_CACHE[key]


def _shard(x: np.ndarray):
    per = BATCH // N_CORES
    return [
        np.ascontiguousarray(x[i * per:(i + 1) * per]).reshape(SLABS, FREE)
        for i in range(N_CORES)
    ]


def _unshard(outs):
    per = BATCH // N_CORES
    return np.concatenate(
        [o.reshape(per, CH, H, W) for o in outs], axis=0
    )


def run(x: np.ndarray, trace: bool = False, **kwargs):
    """Run on hardware; returns (full_output, BassKernelResults)."""
    x = np.asarray(x, dtype=np.float32)
    assert x.shape == (BATCH, CH, H, W), x.shape
    in_maps = [{"x": s} for s in _shard(x)]
    nc = _get_nc()
    res = run_bass_kernel_spmd(
        nc, in_maps, core_ids=list(range(N_CORES)), trace=trace, **kwargs
    )
    out = _unshard([res.results[i]["out"] for i in range(N_CORES)])
    return out, res


def kernel(x) -> np.ndarray:
    out, _ = run(np.asarray(x), trace=False)
    return out


# revision 6
# speedup vs baseline: 1.1930x; 1.1341x over previous
"""BottomPool (cummax along H) for (16, 256, 128, 128) f32 on 8 TRN2 NeuronCores.

Sharding: data-parallel over batch — 2 batches per core. Each core's shard
is viewed as [512 slabs, H*W] where a slab is one (b, c) image of
H=128 x W=128 f32 stored h-major. Partition dim = slab; the cummax along h
runs on the free axis as a two-level blocked scan:
  view free as (B blocks, S h-slices, W), B*S = H
  step1: in-block sequential scan over S   (S-1 vector ops, all blocks at once)
  step2: scan of block-last slices across B (B-1 small ops)
  step3: distribute prev-block prefix into slices 0..S-2 (S-1 ops)
~1.8 vector passes over the data, overlapped with >=1MiB DMAs.

walrus codegen accepts only a small number of sync waits per instruction
(one for DMA pseudo-instructions), but Tile's sem assigner is not
transitively minimal and can attach more. strip_implied_waits() removes
waits that are provably implied: a wait is redundant when the kept waits'
completion closure (instructions that must have completed, including
same-HWDGE-ring FIFO predecessors of completed DMAs) already forces the
waited semaphore to the required value.
"""

import numpy as np

from concourse import bass, mybir, tile
from concourse.bass_utils import run_bass_kernel_spmd

N_CORES = 8
BATCH, CH, H, W = 16, 256, 128, 128
FREE = H * W
P = 128                      # slabs per tile = SBUF partitions
SLABS = (BATCH // N_CORES) * CH  # 512 slabs per core

_NC_CACHE = {}


def _strip_instruction_waits(nc, max_waits={"InstDMACopy": 1}):
    insts = []
    for f in nc.m.functions:
        for b in f.blocks:
            insts.extend(b.instructions)

    # Monotone-sem updater table: sem_id -> [(cum_value_after, inst_idx)].
    # Sems touched by non-monotone updates are excluded entirely.
    poisoned = set()
    cum = {}
    updaters = {}
    inst_updates = [[] for _ in insts]  # idx -> [(sem_id, cum_after)]
    for idx, ins in enumerate(insts):
        si = ins.sync_info
        if si is None:
            continue
        for u in si.on_update:
            if u.update_mode == "sem-add-imm" and u.update_reg is None:
                val = u.update_value
            elif u.update_mode == "sem-inc":
                val = 1
            else:
                poisoned.add(u.id)
                continue
            cum[u.id] = cum.get(u.id, 0) + val
            updaters.setdefault(u.id, []).append((cum[u.id], idx))
            inst_updates[idx].append((u.id, cum[u.id]))

    # Same-HWDGE-ring FIFO order: DMAs issued on one ring complete in
    # program order, so a later DMA's completion implies all earlier ones.
    ring_pos = {}   # inst_idx -> (queue, position)
    ring_members = {}  # queue -> [inst_idx in order]
    for idx, ins in enumerate(insts):
        if isinstance(ins, mybir.InstDMACopy):
            q = ins.queue
            ring_members.setdefault(q, []).append(idx)
            ring_pos[idx] = (q, len(ring_members[q]) - 1)

    inst_waits = []
    for ins in insts:
        si = ins.sync_info
        ws = []
        if si is not None:
            for w in si.on_wait:
                if w.wait_mode == "sem-ge-imm" and w.wait_reg is None:
                    ws.append((w.id, w.wait_value, True))
                else:
                    ws.append((w.id, w.wait_value, False))
        inst_waits.append(ws)

    def facts_from(seed_waits):
        """Fixpoint: semaphore lower bounds guaranteed once seed_waits hold."""
        facts = {}
        for sid, v, clean in seed_waits:
            if clean and sid not in poisoned:
                facts[sid] = max(facts.get(sid, 0), v)
        completed = set()
        changed = True
        while changed:
            changed = False
            for sid, v in list(facts.items()):
                for cval, idx in updaters.get(sid, []):
                    if cval > v:
                        break
                    if idx not in completed:
                        completed.add(idx)
                        changed = True
            for idx in list(completed):
                rp = ring_pos.get(idx)
                if rp is not None:
                    q, pos = rp
                    for pidx in ring_members[q][:pos]:
                        if pidx not in completed:
                            completed.add(pidx)
                            changed = True
            for idx in list(completed):
                for sid, v, clean in inst_waits[idx]:
                    if clean and sid not in poisoned and facts.get(sid, 0) < v:
                        facts[sid] = v
                        changed = True
                for sid, cval in inst_updates[idx]:
                    if sid not in poisoned and facts.get(sid, 0) < cval:
                        facts[sid] = cval
                        changed = True
        return facts

    n_stripped = 0
    for idx, ins in enumerate(insts):
        si = ins.sync_info
        if si is None or len(si.on_wait) <= 1:
            continue
        kept = list(si.on_wait)

        def key(w):
            return (w.id, w.wait_value, w.wait_mode == "sem-ge-imm" and w.wait_reg is None)

        progress = True
        while len(kept) > 1 and progress:
            progress = False
            for w in list(kept):
                sid, v, clean = key(w)
                if not clean or sid in poisoned:
                    continue
                others = [key(k) for k in kept if k is not w]
                if facts_from(others).get(sid, 0) >= v:
                    kept.remove(w)
                    n_stripped += 1
                    progress = True
                    break
        limit = max_waits.get(type(ins).__name__)
        if limit is not None and len(kept) > limit:
            raise RuntimeError(
                f"{type(ins).__name__} {ins.name} still has {len(kept)} waits: "
                f"{[(w.ant_name, w.wait_value) for w in kept]}"
            )
        if len(kept) != len(si.on_wait):
            ins.sync_info = mybir.SyncInfo(on_wait=kept, on_update=list(si.on_update))
    return n_stripped


def build_nc(n_slabs: int = SLABS, bufs: int = 3, blocks: int = 16, halves: int = 2,
             carry_bufs: int = 3, store_engine=lambda nc: nc.gpsimd):
    """halves: split each tile's load/compute/store into this many h-chunks
    (pipelines at finer granularity). The cross-chunk carry (prev chunk's
    running max slice) is copied to a small side tile so a chunk's buffer
    has no readers after its store — keeping every DMA at one sync wait.
    Loads ride the SP HWDGE ring, stores the ACT ring; each ring is FIFO,
    which lets the wait-stripper collapse the kernel-tail drain."""
    B = blocks
    S = H // B
    assert n_slabs % P == 0
    assert B % halves == 0
    n_tiles = n_slabs // P
    BH = B // halves           # blocks per chunk
    CHF = FREE // halves       # free elems per chunk

    nc = bass.Bass("TRN2", target_bir_lowering=False, debug=False)
    x = nc.dram_tensor("x", [n_slabs, FREE], mybir.dt.float32, kind="ExternalInput").ap()
    out = nc.dram_tensor("out", [n_slabs, FREE], mybir.dt.float32, kind="ExternalOutput").ap()

    with tile.TileContext(nc) as tc:
        with tc.tile_pool(name="work", bufs=bufs) as pool, \
             tc.tile_pool(name="carry", bufs=carry_bufs) as cpool:
            for t in range(n_tiles):
                tl = pool.tile([P, FREE], mybir.dt.float32)
                v = tl[:].rearrange("p (b s w) -> p b s w", b=B, s=S, w=W)
                xrow = x[t * P:(t + 1) * P, :]
                orow = out[t * P:(t + 1) * P, :]
                carry = None
                for h in range(halves):
                    b0, b1 = h * BH, (h + 1) * BH
                    c0, c1 = h * CHF, (h + 1) * CHF
                    nc.sync.dma_start(tl[:, c0:c1], xrow[:, c0:c1])
                    # step1: in-block scan, all blocks of this chunk at once
                    for j in range(1, S):
                        nc.vector.tensor_max(
                            v[:, b0:b1, j, :], v[:, b0:b1, j, :], v[:, b0:b1, j - 1, :])
                    # step2: chain block-last slices; chunk h>0 seeds from carry
                    if h:
                        nc.vector.tensor_max(v[:, b0, S - 1, :], v[:, b0, S - 1, :], carry[:])
                    for b in range(b0 + 1, b1):
                        nc.vector.tensor_max(
                            v[:, b, S - 1, :], v[:, b, S - 1, :], v[:, b - 1, S - 1, :])
                    if h + 1 < halves:
                        nxt = cpool.tile([P, W], mybir.dt.float32)
                        nc.vector.tensor_copy(nxt[:], v[:, b1 - 1, S - 1, :])
                    # step3: distribute prev-block prefix into slices 0..S-2
                    for j in range(S - 1):
                        nc.vector.tensor_max(
                            v[:, b0 + 1:b1, j, :], v[:, b0 + 1:b1, j, :],
                            v[:, b0:b1 - 1, S - 1, :])
                    if h:
                        cb = carry[:].unsqueeze(1).broadcast_to([P, S - 1, W])
                        nc.vector.tensor_max(
                            v[:, b0, 0:S - 1, :], v[:, b0, 0:S - 1, :], cb)
                    store_engine(nc).dma_start(orow[:, c0:c1], tl[:, c0:c1])
                    if h + 1 < halves:
                        carry = nxt

    _strip_instruction_waits(nc)
    return nc


def _get_nc():
    key = "default"
    if key not in _NC_CACHE:
        _NC_CACHE[key] = build_nc()
    return _NC_CACHE[key]


def _shard(x: np.ndarray):
    per = BATCH // N_CORES
    return [
        np.ascontiguousarray(x[i * per:(i + 1) * per]).reshape(SLABS, FREE)
        for i in range(N_CORES)
    ]


def _unshard(outs):
    per = BATCH // N_CORES
    return np.concatenate(
        [o.reshape(per, CH, H, W) for o in outs], axis=0
    )


def run(x: np.ndarray, trace: bool = False, **kwargs):
    """Run on hardware; returns (full_output, BassKernelResults)."""
    x = np.asarray(x, dtype=np.float32)
    assert x.shape == (BATCH, CH, H, W), x.shape
    in_maps = [{"x": s} for s in _shard(x)]
    nc = _get_nc()
    res = run_bass_kernel_spmd(
        nc, in_maps, core_ids=list(range(N_CORES)), trace=trace, **kwargs
    )
    out = _unshard([res.results[i]["out"] for i in range(N_CORES)])
    return out, res


def kernel(x) -> np.ndarray:
    out, _ = run(np.asarray(x), trace=False)
    return out
